# revision 23
# baseline (speedup 1.0000x reference)
"""Trainium2 Bass kernel for nn_DimCosSoftmaxModule (8-core SPMD).

Math (exact refactor of the reference):
  k1[n,j,t] = relu(sum_i mem_feat[n,i] wt[i,j,t] + bt[j])                 [200,2048,3]
  k2[n,o,s] = relu(sum_{i,dt} wc[o,i,dt] k1pad[n,i,s+dt-1] + bc[o])      [200,2048,3]
  conv/sp_down fold: cls[b,n] = sum_{i,t} G[b,i,t] k2[n,i,t] + b_sp
      where G[b,i,t] = sum_u feat[b,i,u] V[u,t],  V = shifted copies of w_sp
  out = 30*(cosine(cls, w_cls) - 0.5*onehot(label))

Sharding: tensor-parallel over the 2048 channel dim (256 ch/core).
  step1 column-sharded -> single AllGather of k1 (bf16, Shared output)
  -> step2 o-sharded -> local partial cls -> AllToAll (mesh, ~8us) +
  DVE tree-sum so core c keeps batch rows 8c..8c+8 -> row-local
  CosFace.  Host reassembles the 8 row-shards of y.

Measured constraints this schedule is built around: no collective
transfer starts before the device-wide collective-init barrier ends
(~60us) plus ~11.5us first-op setup; a single RDH AllGather moves
2.45MB in ~28us (fastest per byte of AG/RS/AllToAll here); the tail
AllToAll+local-sum replaces a ReduceScatter at half the latency.

All bulk inputs are host-restaged to fully-contiguous [128, L] partition
layouts so every HWDGE DMA moves multi-KB runs per partition (line rate).
"""
import os
os.environ.setdefault("NEURON_RT_DBG_RDH_CC", "0")

import numpy as np
import ml_dtypes

import concourse.bass as bass
import concourse.bacc as bacc
import concourse.mybir as mybir
import concourse.tile as tile
from concourse import bass_utils
from concourse.masks import make_identity

N_CORES = 8
BS, C, HW = 64, 2048, 196
NM = 200                 # N_MEM == NUM_CLASSES
SH = C // N_CORES        # 256 channels per core
NIT = C // 128           # 16 i-tiles of 128
NOT = C // 128           # 16 o-tiles of 128 (step2 partial covers all o)
RB = BS // N_CORES       # rows per core after the final exchange
S_SCALE, M_MARGIN = 30.0, 0.5

BF16 = mybir.dt.bfloat16
FP16 = mybir.dt.float16
F32 = mybir.dt.float32
AF = mybir.ActivationFunctionType
ALU = mybir.AluOpType

NX = BS * SH             # 16384 G columns per core
NQ = NX // 128           # 128 q-groups

# x-column chunk boundaries for ft/G pipelining (3 chunks per u-half)
FT_SPLITS = [0, 43 * 128, 86 * 128, NX]

TRACE = False
TRACE_KW = {}
LAST_RESULT = None
_CACHE = {}


def build_nc():
    nc = bacc.Bacc("TRN2", target_bir_lowering=False, debug=False, num_devices=N_CORES)

    # per-core external inputs; every bulk tensor is staged host-side so the
    # DMA is identity-contiguous per partition.
    MFT = nc.dram_tensor("mft", [128, NIT, NM], BF16, kind="ExternalInput")
    WT = nc.dram_tensor("wtc", [128, 2, 3, NIT, 128], BF16, kind="ExternalInput")
    WCT = nc.dram_tensor("wct", [128, NIT, 3, 2, 128], BF16, kind="ExternalInput")
    FT0 = nc.dram_tensor("ft0", [128, NX], BF16, kind="ExternalInput")
    FT1 = nc.dram_tensor("ft1", [68, NX], BF16, kind="ExternalInput")
    VM = nc.dram_tensor("vm", [128, 2, 3], BF16, kind="ExternalInput")
    SM = nc.dram_tensor("sm", [128, 4], F32, kind="ExternalInput")     # bt | bc
    WCLS = nc.dram_tensor("wcls", [128, 2, NM], F32, kind="ExternalInput")
    EP = nc.dram_tensor("ep", [RB, NM + 2], F32, kind="ExternalInput") # iota | lbl | bsp
    Y = nc.dram_tensor("y", [RB, NM], F32, kind="ExternalOutput")

    with tile.TileContext(nc) as tc:
        with (
            tc.tile_pool(name="sbuf", bufs=1) as sbuf,
            tc.tile_pool(name="psum", bufs=1, space="PSUM") as psum,
            tc.tile_pool(name="dram", bufs=1, space="DRAM") as dram,
        ):
            # ---------------- input DMAs ----------------
            # sync(SP) ring:   mft -> wt jc1 -> wct ot 0:8 -> ft1 chunks
            # scalar(ACT) ring: wt jc0 -> wct ot 8:16 -> smalls -> ft0 chunks
            # The FIRST 8 DMAs emitted map 1:1 onto the 8 HWDGE semaphore
            # lanes; later DMAs alias those lanes.  Keeping every step-1/2
            # critical input in the first 8 means no critical wait ever
            # aliases a big late-finishing ft transfer.
            mf_sb = sbuf.tile([128, NIT, NM], BF16, tag="mf")
            nc.sync.dma_start(mf_sb[:], MFT[:])
            # all small tensors lead the scalar ring: they gate cheap DVE/ACT
            # ops (relu biases, masks) whose queue position would otherwise
            # head-of-line-block the whole engine behind bulk transfers
            sm_sb = sbuf.tile([128, 4], F32, tag="sm")
            nc.scalar.dma_start(sm_sb[:], SM[:])
            v_sb = sbuf.tile([128, 2, 3], BF16, tag="v")
            nc.scalar.dma_start(v_sb[:], VM[:])
            wcls_sb = sbuf.tile([128, 2, NM], F32, tag="wcls")
            nc.scalar.dma_start(wcls_sb[:], WCLS[:])
            ep_sb = sbuf.tile([RB, NM + 2], F32, tag="ep")
            nc.scalar.dma_start(ep_sb[:], EP[:])
            wt_sb = sbuf.tile([128, 2, 3, NIT, 128], BF16, tag="wt")
            nc.scalar.dma_start(wt_sb[:, 0], WT[:, 0])
            nc.sync.dma_start(wt_sb[:, 1], WT[:, 1])

            # k1 bounce buffers ride the rings BEFORE wct/ft so the AllGather
            # halves trigger as soon as step 1 finishes
            k1_sb = sbuf.tile([128, 2, 3, NM], BF16, tag="k1")
            kb = dram.tile([SH, 3, NM], BF16, name="k1_bounce")
            kg = dram.tile([C, 3, NM], BF16, name="k1_gath", addr_space="Shared")
            wcT_sb = sbuf.tile([128, NIT, 3, 2, 128], BF16, tag="wcT")
            nc.sync.dma_start(wcT_sb[:, 0:8], WCT[:, 0:8])
            nc.scalar.dma_start(wcT_sb[:, 8:16], WCT[:, 8:16])

            ft0_sb = sbuf.tile([128, NX], BF16, tag="ft0")
            ft1_sb = sbuf.tile([68, NX], BF16, tag="ft1")

            # ---------------- constants ----------------
            idn = sbuf.tile([128, 128], F32, tag="idn")
            make_identity(nc, idn[:])
            ones1 = sbuf.tile([1, RB], F32, tag="ones1")
            nc.vector.memset(ones1[:], 1.0)
            onesc = sbuf.tile([128, 1], F32, tag="onesc")
            nc.vector.memset(onesc[:], 1.0)

            # ---------------- step 1 + pipelined AllGather halves ----------------
            # jc-outer so each 128-wide j-half is gathered while the other
            # half computes.  Gathered half jc holds global j-tiles {2c + jc}.
            for jc in range(2):
                for t in range(3):
                    ps1 = psum.tile([128, NM], F32, tag="ps1", bufs=2, name=f"ps1_{jc}_{t}")
                    for it in range(NIT):
                        nc.tensor.matmul(
                            ps1[:],
                            wt_sb[:, jc, t, it, :],
                            mf_sb[:, it, :],
                            start=(it == 0), stop=(it == NIT - 1),
                        )
                    nc.vector.tensor_scalar(k1_sb[:, jc, t, :], ps1[:],
                                            sm_sb[:, jc:jc + 1], 0.0,
                                            ALU.add, ALU.max)
            nc.sync.dma_start(kb.rearrange("(jc p) t n -> p jc t n", p=128), k1_sb[:])
            nc.gpsimd.collective_compute(
                "AllGather", ALU.bypass,
                replica_groups=[list(range(N_CORES))],
                ins=[kb.opt()], outs=[kg.opt()],
            )

            # ft (for G) and the k1f reloads, emitted after the AG trigger so
            # the reloads' dependency on the gathered output is tracked
            for a, b in zip(FT_SPLITS[:-1], FT_SPLITS[1:]):
                nc.sync.dma_start(ft1_sb[:, a:b], FT1[:, a:b])
                nc.sync.dma_start(ft0_sb[:, a:b], FT0[:, a:b])
            k1f = sbuf.tile([128, 2, N_CORES, 3, NM], BF16, tag="k1f")
            kgv = kg.rearrange("(c jc p) t n -> p jc c t n", p=128, jc=2)
            nc.scalar.dma_start(k1f[:, 0], kgv[:, 0])
            nc.scalar.dma_start(k1f[:, 1], kgv[:, 1])

            # ---------------- G: featT-stationary matmuls ----------------
            # out[x-chunk, t] = sum_u featT[u, x] V[u, t]; 42 chunks packed per
            # PSUM bank, one DVE cast-copy per bank into gbuf16.
            # gbuf16 free index = 3*q + t with q = chunk = b*2 + h.
            gbuf16 = sbuf.tile([128, NQ * 3], BF16, tag="gbuf16")
            CPB = 42                       # chunks per bank
            nbanks = (NQ + CPB - 1) // CPB
            for bank in range(nbanks):
                c0 = bank * CPB
                c1 = min(c0 + CPB, NQ)
                gpk = psum.tile([128, CPB * 3], F32, tag="gpk", bufs=2, name=f"gpk{bank}")
                for c in range(c0, c1):
                    col = (c - c0) * 3
                    nc.tensor.matmul(gpk[:, col:col + 3],
                                     ft0_sb[:, c * 128:(c + 1) * 128],
                                     v_sb[0:128, 0, :], start=True, stop=False)
                    nc.tensor.matmul(gpk[:, col:col + 3],
                                     ft1_sb[0:68, c * 128:(c + 1) * 128],
                                     v_sb[0:68, 1, :], start=False, stop=True)
                nc.vector.tensor_copy(gbuf16[:, c0 * 3:c1 * 3], gpk[:, 0:(c1 - c0) * 3])

            # ---------------- CosFace precompute (overlaps collective) ----------------
            # S / |w_cls_col| broadcast to [RB, NM], and S*M*onehot mask.
            wsq_sb = sbuf.tile([128, 2, NM], F32, tag="wsq")
            nc.scalar.activation(wsq_sb[0:128, 0, :], wcls_sb[0:128, 0, :], AF.Square)
            nc.scalar.activation(wsq_sb[0:72, 1, :], wcls_sb[0:72, 1, :], AF.Square)
            wsA = psum.tile([128, 1], F32, tag="ep", name="wsA")
            nc.tensor.matmul(wsA[:], wsq_sb[0:128, 0, 0:128], onesc[0:128, :], start=True, stop=False)
            nc.tensor.matmul(wsA[:], wsq_sb[0:72, 1, 0:128], onesc[0:72, :], start=False, stop=True)
            wsB = psum.tile([72, 1], F32, tag="ep", name="wsB")
            nc.tensor.matmul(wsB[:], wsq_sb[0:128, 0, 128:NM], onesc[0:128, :], start=True, stop=False)
            nc.tensor.matmul(wsB[:], wsq_sb[0:72, 1, 128:NM], onesc[0:72, :], start=False, stop=True)
            wnorm_sb = sbuf.tile([128, 2], F32, tag="wnorm")
            nc.scalar.activation(wnorm_sb[:, 0:1], wsA[:], AF.Sqrt)
            nc.scalar.activation(wnorm_sb[0:72, 1:2], wsB[:], AF.Sqrt)
            winv_sb = sbuf.tile([128, 2], F32, tag="winv")
            nc.vector.reciprocal(winv_sb[:, 0:1], wnorm_sb[:, 0:1])
            nc.vector.reciprocal(winv_sb[0:72, 1:2], wnorm_sb[0:72, 1:2])
            winvrow_sb = sbuf.tile([1, NM], F32, tag="winvrow")
            wr1 = psum.tile([1, 128], F32, tag="ep", name="wr1")
            nc.tensor.transpose(wr1[:], winv_sb[:, 0:1], idn[:])
            nc.vector.tensor_copy(winvrow_sb[:, 0:128], wr1[:])
            wr2 = psum.tile([1, 72], F32, tag="ep", name="wr2")
            nc.tensor.transpose(wr2[:], winv_sb[0:72, 1:2], idn[0:72, 0:72])
            nc.vector.tensor_copy(winvrow_sb[:, 128:NM], wr2[:])
            wbps = psum.tile([RB, NM], F32, tag="ep", name="wbps")
            nc.tensor.matmul(wbps[:], ones1[:], winvrow_sb[:], start=True, stop=True)
            winvbS_sb = sbuf.tile([RB, NM], F32, tag="winvbS")
            nc.scalar.mul(winvbS_sb[:], wbps[:], S_SCALE)
            maskSM_sb = sbuf.tile([RB, NM], F32, tag="maskSM")
            nc.vector.tensor_scalar(maskSM_sb[:], ep_sb[:, 0:NM],
                                    ep_sb[:, NM:NM + 1], None, ALU.is_equal)
            nc.vector.tensor_scalar(maskSM_sb[:], maskSM_sb[:], S_SCALE * M_MARGIN,
                                    None, ALU.mult)

            # ---------------- step 2: k2T_s[o, n] for the local o-slice ----------------
            k2_sb = sbuf.tile([128, 2, 3, NM], BF16, tag="k2")
            for oc in range(2):
                # bank A holds s=0,1 (N=400), bank B holds s=2 (N=200)
                psA = psum.tile([128, 2 * NM], F32, tag="ps2A", bufs=2, name=f"ps2A_{oc}")
                psB = psum.tile([128, NM], F32, tag="ps2B", bufs=1, name=f"ps2B_{oc}")
                n_it = 0
                for h in range(2):          # gathered half h first
                    for g in range(N_CORES):
                        it = 2 * g + h      # global 128-j tile index
                        first = (n_it == 0)
                        last = (n_it == 2 * N_CORES - 1)
                        kv = k1f[:, h].rearrange("p c t n -> p c (t n)")
                        l0 = wcT_sb[:, it, 0, oc, :]
                        l1 = wcT_sb[:, it, 1, oc, :]
                        l2 = wcT_sb[:, it, 2, oc, :]
                        # dt=1: t'=0,1 -> s=0,1 (A[0:400])
                        nc.tensor.matmul(psA[:, 0:2 * NM], l1, kv[:, g, 0:2 * NM],
                                         start=first, stop=False)
                        # dt=0: t'=0 -> s=1 (A[200:400])
                        nc.tensor.matmul(psA[:, NM:2 * NM], l0, kv[:, g, 0:NM],
                                         start=False, stop=False)
                        # dt=2: t'=1,2 -> s=0,1 (A[0:400])
                        nc.tensor.matmul(psA[:, 0:2 * NM], l2, kv[:, g, NM:3 * NM],
                                         start=False, stop=last)
                        # dt=0: t'=1 -> s=2 (B)
                        nc.tensor.matmul(psB[:], l0, kv[:, g, NM:2 * NM],
                                         start=first, stop=False)
                        # dt=1: t'=2 -> s=2 (B)
                        nc.tensor.matmul(psB[:], l1, kv[:, g, 2 * NM:3 * NM],
                                         start=False, stop=last)
                        n_it += 1
                nc.vector.tensor_scalar(k2_sb[:, oc, 0, :], psA[:, 0:NM],
                                        sm_sb[:, 2 + oc:3 + oc], 0.0, ALU.add, ALU.max)
                nc.vector.tensor_scalar(k2_sb[:, oc, 1, :], psA[:, NM:2 * NM],
                                        sm_sb[:, 2 + oc:3 + oc], 0.0, ALU.add, ALU.max)
                nc.vector.tensor_scalar(k2_sb[:, oc, 2, :], psB[:],
                                        sm_sb[:, 2 + oc:3 + oc], 0.0, ALU.add, ALU.max)

            # ---------------- cls partial: [64, 200] ----------------
            cps = psum.tile([BS, NM], F32, tag="ep", name="cps")
            first = True
            for h in range(2):
                for t in range(3):
                    lhs = gbuf16[:, 3 * h + t::6]
                    nc.tensor.matmul(cps[:], lhs[:, 0:BS], k2_sb[:, h, t, :],
                                     start=first, stop=(h == 1 and t == 2))
                    first = False
            clsp_sb = sbuf.tile([BS, NM], F32, tag="clsp")
            nc.vector.tensor_copy(clsp_sb[:], cps[:])

            # ---------------- AllToAll + local tree reduce ----------------
            # Core c ends with the full 8-way sum of batch rows 8c..8c+8.
            cls_bounce = dram.tile([BS, NM], F32, name="cls_bounce")
            cls_x = dram.tile([BS, NM], F32, name="cls_x")
            nc.scalar.dma_start(cls_bounce[:], clsp_sb[:])
            nc.gpsimd.collective_compute(
                "AllToAll", ALU.bypass,
                replica_groups=[list(range(N_CORES))],
                ins=[cls_bounce.opt()], outs=[cls_x.opt()],
            )
            xs_sb = sbuf.tile([RB, N_CORES, NM], F32, tag="xs")
            nc.sync.dma_start(xs_sb[:], cls_x.rearrange("(s p) n -> p s n", p=RB))
            t4_sb = sbuf.tile([RB, 4, NM], F32, tag="t4")
            nc.vector.tensor_tensor(t4_sb[:], xs_sb[:, 0:4], xs_sb[:, 4:8], ALU.add)
            t2_sb = sbuf.tile([RB, 2, NM], F32, tag="t2")
            nc.vector.tensor_tensor(t2_sb[:], t4_sb[:, 0:2], t4_sb[:, 2:4], ALU.add)
            cls_sb = sbuf.tile([RB, NM], F32, tag="cls")
            # fold b_sp into the last tree level: (a + b_sp) + b
            nc.vector.scalar_tensor_tensor(cls_sb[:], t2_sb[:, 0],
                                           ep_sb[:, NM + 1:NM + 2], t2_sb[:, 1],
                                           ALU.add, ALU.add)

            # ---------------- CosFace epilogue ----------------
            # norm path (ACT) and transpose/cos path (PE) run in parallel.
            sq_sb = sbuf.tile([RB, NM], F32, tag="sq")
            ss_sb = sbuf.tile([RB, 1], F32, tag="ss")
            nc.scalar.activation(sq_sb[:], cls_sb[:], AF.Square, accum_out=ss_sb[:])
            rt_sb = sbuf.tile([RB, 1], F32, tag="rt")
            nc.scalar.activation(rt_sb[:], ss_sb[:], AF.Sqrt)
            invx_sb = sbuf.tile([RB, 1], F32, tag="invx")
            nc.vector.reciprocal(invx_sb[:], rt_sb[:])
            clsT_sb = sbuf.tile([128, 2, RB], F32, tag="clsT")
            tp1 = psum.tile([128, RB], F32, tag="ep", name="tp1")
            nc.tensor.transpose(tp1[:], cls_sb[:, 0:128], idn[0:RB, 0:RB])
            nc.vector.tensor_copy(clsT_sb[0:128, 0, :], tp1[:])
            tp2 = psum.tile([72, RB], F32, tag="ep", name="tp2")
            nc.tensor.transpose(tp2[:], cls_sb[:, 128:NM], idn[0:RB, 0:RB])
            nc.vector.tensor_copy(clsT_sb[0:72, 1, :], tp2[:])
            cos_ps = psum.tile([RB, NM], F32, tag="ep", name="cos_ps")
            nc.tensor.matmul(cos_ps[:], clsT_sb[0:128, 0, :], wcls_sb[0:128, 0, :],
                             start=True, stop=False)
            nc.tensor.matmul(cos_ps[:], clsT_sb[0:72, 1, :], wcls_sb[0:72, 1, :],
                             start=False, stop=True)
            t1_sb = sbuf.tile([RB, NM], F32, tag="t1")
            nc.vector.scalar_tensor_tensor(t1_sb[:], cos_ps[:], invx_sb[:],
                                           winvbS_sb[:], ALU.mult, ALU.mult)
            out_sb = sbuf.tile([RB, NM], F32, tag="out")
            nc.vector.tensor_tensor(out_sb[:], t1_sb[:], maskSM_sb[:], ALU.subtract)
            nc.sync.dma_start(Y[:], out_sb[:])

    nc.compile()
    return nc


def _prep_inputs(feat, label, mem_feat, wt, bt, wc, bc, w_sp, b_sp, w_cls):
    bf = ml_dtypes.bfloat16
    f32 = np.float32
    feat = np.ascontiguousarray(np.asarray(feat, dtype=f32))
    mem_feat = np.asarray(mem_feat, dtype=f32)
    wt = np.asarray(wt, dtype=f32)
    bt = np.asarray(bt, dtype=f32)
    wc = np.asarray(wc, dtype=f32)
    bc = np.asarray(bc, dtype=f32)
    w_sp = np.asarray(w_sp, dtype=f32)
    b_sp = np.asarray(b_sp, dtype=f32)
    w_cls = np.asarray(w_cls, dtype=f32)
    label = np.asarray(label)

    # V[u, t]: shifted copies of w_sp so conv+sp_down folds into G
    V = np.zeros((HW, 3), f32)
    V[:HW - 1, 0] = w_sp[0, 1:]
    V[:, 1] = w_sp[0, :]
    V[1:, 2] = w_sp[0, :HW - 1]
    vm = np.zeros((128, 2, 3), f32)
    vm[0:128, 0, :] = V[0:128]
    vm[0:68, 1, :] = V[128:HW]
    vm = vm.astype(bf)

    # mft[p, it, n] = mem_feat.T[it*128+p, n]
    mft = np.ascontiguousarray(
        mem_feat.T.reshape(NIT, 128, NM).transpose(1, 0, 2)).astype(bf)

    # wcls[p, half, n]: wclsT rows 0:128 | rows 128:200 (zero-padded)
    wclsT = np.ascontiguousarray(w_cls.T)          # [200, 200] f32
    wcls = np.zeros((128, 2, NM), f32)
    wcls[0:128, 0] = wclsT[0:128]
    wcls[0:72, 1] = wclsT[128:NM]

    iota = np.arange(NM, dtype=f32)

    fv = feat.reshape(BS, C, HW)
    in_maps = []
    for c in range(N_CORES):
        J = slice(c * SH, (c + 1) * SH)
        # wt[p, jc, t, it, jl] = wt[it*128+p, c*256+jc*128+jl, t]
        wt_c = np.ascontiguousarray(
            wt[:, J, :].reshape(NIT, 128, 2, 128, 3).transpose(1, 2, 4, 0, 3)
        ).astype(bf)
        # wct[p, it, dt, oc, ol] = wc[c*256+oc*128+ol, it*128+p, dt]
        wc_c = np.ascontiguousarray(
            wc[J, :, :].reshape(2, 128, NIT, 128, 3).transpose(3, 2, 4, 0, 1)
        ).astype(bf)
        # featT slice: [u, x] with x = b*256 + i_local
        ft_c = np.ascontiguousarray(
            fv[:, J, :].transpose(2, 0, 1).reshape(HW, NX)).astype(bf)
        ft0 = np.ascontiguousarray(ft_c[0:128])
        ft1 = np.zeros((68, NX), f32).astype(bf)
        ft1[0:68] = ft_c[128:HW]
        sm = np.zeros((128, 4), f32)
        sm[:, 0] = bt[J][0:128]
        sm[:, 1] = bt[J][128:256]
        sm[:, 2] = bc[J][0:128]
        sm[:, 3] = bc[J][128:256]
        ep = np.zeros((RB, NM + 2), f32)
        ep[:, 0:NM] = iota[None, :]
        ep[:, NM] = label[c * RB:(c + 1) * RB].astype(f32)
        ep[:, NM + 1] = b_sp[0]
        in_maps.append({
            "mft": mft, "wtc": wt_c, "wct": wc_c,
            "ft0": ft0, "ft1": ft1, "vm": vm,
            "sm": sm, "wcls": wcls, "ep": ep,
        })
    return in_maps


def kernel(**inputs) -> np.ndarray:
    global LAST_RESULT
    if "nc" not in _CACHE:
        _CACHE["nc"] = build_nc()
    nc = _CACHE["nc"]
    in_maps = _prep_inputs(**inputs)
    try:
        res = bass_utils.run_bass_kernel_spmd(
            nc, in_maps, core_ids=list(range(N_CORES)),
            trace=TRACE, **TRACE_KW,
        )
    except Exception:
        # transient NRT/device hiccups recover on retry
        res = bass_utils.run_bass_kernel_spmd(
            nc, in_maps, core_ids=list(range(N_CORES)),
            trace=TRACE, **TRACE_KW,
        )
    LAST_RESULT = res
    return np.concatenate(
        [np.asarray(res.results[c]["y"], dtype=np.float32) for c in range(N_CORES)],
        axis=0,
    )


# revision 24
# speedup vs baseline: 1.0153x; 1.0153x over previous
"""Trainium2 Bass kernel for nn_DimCosSoftmaxModule (8-core SPMD).

Math (exact refactor of the reference):
  k1[n,j,t] = relu(sum_i mem_feat[n,i] wt[i,j,t] + bt[j])                 [200,2048,3]
  k2[n,o,s] = relu(sum_{i,dt} wc[o,i,dt] k1pad[n,i,s+dt-1] + bc[o])      [200,2048,3]
  conv/sp_down fold: cls[b,n] = sum_{i,t} G[b,i,t] k2[n,i,t] + b_sp
      where G[b,i,t] = sum_u feat[b,i,u] V[u,t],  V = shifted copies of w_sp
  out = 30*(cosine(cls, w_cls) - 0.5*onehot(label))

Sharding: tensor-parallel over the 2048 channel dim (256 ch/core).
  step1 column-sharded -> single AllGather of k1 (bf16, Shared output)
  -> step2 o-sharded -> local partial cls -> AllToAll (mesh, ~8us) +
  DVE tree-sum so core c keeps batch rows 8c..8c+8 -> row-local
  CosFace.  Host reassembles the 8 row-shards of y.

Measured constraints this schedule is built around: no collective
transfer starts before the device-wide collective-init barrier ends
(~60us) plus ~11.5us first-op setup; a single RDH AllGather moves
2.45MB in ~28us (fastest per byte of AG/RS/AllToAll here); the tail
AllToAll+local-sum replaces a ReduceScatter at half the latency.

All bulk inputs are host-restaged to fully-contiguous [128, L] partition
layouts so every HWDGE DMA moves multi-KB runs per partition (line rate).
"""
import os
os.environ.setdefault("NEURON_RT_DBG_RDH_CC", "0")

import numpy as np
import ml_dtypes

import concourse.bass as bass
import concourse.bacc as bacc
import concourse.mybir as mybir
import concourse.tile as tile
from concourse import bass_utils
from concourse.masks import make_identity

N_CORES = 8
BS, C, HW = 64, 2048, 196
NM = 200                 # N_MEM == NUM_CLASSES
SH = C // N_CORES        # 256 channels per core
NIT = C // 128           # 16 i-tiles of 128
NOT = C // 128           # 16 o-tiles of 128 (step2 partial covers all o)
RB = BS // N_CORES       # rows per core after the final exchange
S_SCALE, M_MARGIN = 30.0, 0.5

BF16 = mybir.dt.bfloat16
FP16 = mybir.dt.float16
F32 = mybir.dt.float32
AF = mybir.ActivationFunctionType
ALU = mybir.AluOpType

NX = BS * SH             # 16384 G columns per core
NQ = NX // 128           # 128 q-groups

# x-column chunk boundaries for ft/G pipelining (3 chunks per u-half)
FT_SPLITS = [0, 43 * 128, 86 * 128, NX]

TRACE = False
TRACE_KW = {}
LAST_RESULT = None
_CACHE = {}


def build_nc():
    nc = bacc.Bacc("TRN2", target_bir_lowering=False, debug=False, num_devices=N_CORES)

    # per-core external inputs; every bulk tensor is staged host-side so the
    # DMA is identity-contiguous per partition.
    MFT = nc.dram_tensor("mft", [128, NIT, NM], BF16, kind="ExternalInput")
    WT = nc.dram_tensor("wtc", [128, 2, 3, NIT, 128], BF16, kind="ExternalInput")
    WCT = nc.dram_tensor("wct", [128, NIT, 3, 2, 128], BF16, kind="ExternalInput")
    FT0 = nc.dram_tensor("ft0", [128, NX], BF16, kind="ExternalInput")
    FT1 = nc.dram_tensor("ft1", [68, NX], BF16, kind="ExternalInput")
    VM = nc.dram_tensor("vm", [128, 2, 3], BF16, kind="ExternalInput")
    SM = nc.dram_tensor("sm", [128, 4], F32, kind="ExternalInput")     # bt | bc
    WCLS = nc.dram_tensor("wcls", [128, 2, NM], F32, kind="ExternalInput")
    EP = nc.dram_tensor("ep", [RB, NM + 2], F32, kind="ExternalInput") # iota | lbl | bsp
    Y = nc.dram_tensor("y", [RB, NM], F32, kind="ExternalOutput")

    with tile.TileContext(nc) as tc:
        with (
            tc.tile_pool(name="sbuf", bufs=1) as sbuf,
            tc.tile_pool(name="psum", bufs=1, space="PSUM") as psum,
            tc.tile_pool(name="dram", bufs=1, space="DRAM") as dram,
        ):
            # ---------------- input DMAs ----------------
            # sync(SP) ring:   mft -> wt jc1 -> wct ot 0:8 -> ft1 chunks
            # scalar(ACT) ring: wt jc0 -> wct ot 8:16 -> smalls -> ft0 chunks
            # The FIRST 8 DMAs emitted map 1:1 onto the 8 HWDGE semaphore
            # lanes; later DMAs alias those lanes.  Keeping every step-1/2
            # critical input in the first 8 means no critical wait ever
            # aliases a big late-finishing ft transfer.
            mf_sb = sbuf.tile([128, NIT, NM], BF16, tag="mf")
            nc.sync.dma_start(mf_sb[:], MFT[:])
            # all small tensors lead the scalar ring: they gate cheap DVE/ACT
            # ops (relu biases, masks) whose queue position would otherwise
            # head-of-line-block the whole engine behind bulk transfers
            sm_sb = sbuf.tile([128, 4], F32, tag="sm")
            nc.scalar.dma_start(sm_sb[:], SM[:])
            v_sb = sbuf.tile([128, 2, 3], BF16, tag="v")
            nc.scalar.dma_start(v_sb[:], VM[:])
            wcls_sb = sbuf.tile([128, 2, NM], F32, tag="wcls")
            nc.scalar.dma_start(wcls_sb[:], WCLS[:])
            ep_sb = sbuf.tile([RB, NM + 2], F32, tag="ep")
            nc.scalar.dma_start(ep_sb[:], EP[:])
            wt_sb = sbuf.tile([128, 2, 3, NIT, 128], BF16, tag="wt")
            nc.scalar.dma_start(wt_sb[:, 0], WT[:, 0])
            nc.sync.dma_start(wt_sb[:, 1], WT[:, 1])

            # k1 bounce buffers ride the rings BEFORE wct/ft so the AllGather
            # halves trigger as soon as step 1 finishes
            k1_sb = sbuf.tile([128, 2, 3, NM], BF16, tag="k1")
            kb = dram.tile([SH, 3, NM], BF16, name="k1_bounce")
            kg = dram.tile([C, 3, NM], BF16, name="k1_gath", addr_space="Shared")
            wcT_sb = sbuf.tile([128, NIT, 3, 2, 128], BF16, tag="wcT")
            nc.sync.dma_start(wcT_sb[:, 0:8], WCT[:, 0:8])
            nc.scalar.dma_start(wcT_sb[:, 8:16], WCT[:, 8:16])

            ft0_sb = sbuf.tile([128, NX], BF16, tag="ft0")
            ft1_sb = sbuf.tile([68, NX], BF16, tag="ft1")

            # ---------------- constants ----------------
            idn = sbuf.tile([128, 128], F32, tag="idn")
            make_identity(nc, idn[:])
            ones1 = sbuf.tile([1, RB], F32, tag="ones1")
            nc.vector.memset(ones1[:], 1.0)
            onesc = sbuf.tile([128, 1], F32, tag="onesc")
            nc.vector.memset(onesc[:], 1.0)

            # ---------------- step 1 + pipelined AllGather halves ----------------
            # jc-outer so each 128-wide j-half is gathered while the other
            # half computes.  Gathered half jc holds global j-tiles {2c + jc}.
            for jc in range(2):
                for t in range(3):
                    ps1 = psum.tile([128, NM], F32, tag="ps1", bufs=1, name=f"ps1_{jc}_{t}")
                    for it in range(NIT):
                        nc.tensor.matmul(
                            ps1[:],
                            wt_sb[:, jc, t, it, :],
                            mf_sb[:, it, :],
                            start=(it == 0), stop=(it == NIT - 1),
                        )
                    nc.vector.tensor_scalar(k1_sb[:, jc, t, :], ps1[:],
                                            sm_sb[:, jc:jc + 1], 0.0,
                                            ALU.add, ALU.max)
            nc.sync.dma_start(kb.rearrange("(jc p) t n -> p jc t n", p=128), k1_sb[:])
            nc.gpsimd.collective_compute(
                "AllGather", ALU.bypass,
                replica_groups=[list(range(N_CORES))],
                ins=[kb.opt()], outs=[kg.opt()],
            )

            # ft (for G) and the k1f reloads, emitted after the AG trigger so
            # the reloads' dependency on the gathered output is tracked
            for a, b in zip(FT_SPLITS[:-1], FT_SPLITS[1:]):
                nc.sync.dma_start(ft1_sb[:, a:b], FT1[:, a:b])
                nc.sync.dma_start(ft0_sb[:, a:b], FT0[:, a:b])
            k1f = sbuf.tile([128, 2, N_CORES, 3, NM], BF16, tag="k1f")
            kgv = kg.rearrange("(c jc p) t n -> p jc c t n", p=128, jc=2)
            nc.sync.dma_start(k1f[:, 0], kgv[:, 0])
            nc.scalar.dma_start(k1f[:, 1], kgv[:, 1])

            # ---------------- G: featT-stationary matmuls ----------------
            # out[x-chunk, t] = sum_u featT[u, x] V[u, t]; 42 chunks packed per
            # PSUM bank, one DVE cast-copy per bank into gbuf16.
            # gbuf16 free index = 3*q + t with q = chunk = b*2 + h.
            gbuf16 = sbuf.tile([128, NQ * 3], BF16, tag="gbuf16")
            CPB = 42                       # chunks per bank
            nbanks = (NQ + CPB - 1) // CPB
            for bank in range(nbanks):
                c0 = bank * CPB
                c1 = min(c0 + CPB, NQ)
                gpk = psum.tile([128, CPB * 3], F32, tag="gpk", bufs=2, name=f"gpk{bank}")
                for c in range(c0, c1):
                    col = (c - c0) * 3
                    nc.tensor.matmul(gpk[:, col:col + 3],
                                     ft0_sb[:, c * 128:(c + 1) * 128],
                                     v_sb[0:128, 0, :], start=True, stop=False)
                    nc.tensor.matmul(gpk[:, col:col + 3],
                                     ft1_sb[0:68, c * 128:(c + 1) * 128],
                                     v_sb[0:68, 1, :], start=False, stop=True)
                nc.vector.tensor_copy(gbuf16[:, c0 * 3:c1 * 3], gpk[:, 0:(c1 - c0) * 3])

            # ---------------- CosFace precompute (overlaps collective) ----------------
            # S / |w_cls_col| broadcast to [RB, NM], and S*M*onehot mask.
            wsq_sb = sbuf.tile([128, 2, NM], F32, tag="wsq")
            nc.scalar.activation(wsq_sb[0:128, 0, :], wcls_sb[0:128, 0, :], AF.Square)
            nc.scalar.activation(wsq_sb[0:72, 1, :], wcls_sb[0:72, 1, :], AF.Square)
            wsA = psum.tile([128, 1], F32, tag="ep", name="wsA")
            nc.tensor.matmul(wsA[:], wsq_sb[0:128, 0, 0:128], onesc[0:128, :], start=True, stop=False)
            nc.tensor.matmul(wsA[:], wsq_sb[0:72, 1, 0:128], onesc[0:72, :], start=False, stop=True)
            wsB = psum.tile([72, 1], F32, tag="ep", name="wsB")
            nc.tensor.matmul(wsB[:], wsq_sb[0:128, 0, 128:NM], onesc[0:128, :], start=True, stop=False)
            nc.tensor.matmul(wsB[:], wsq_sb[0:72, 1, 128:NM], onesc[0:72, :], start=False, stop=True)
            wnorm_sb = sbuf.tile([128, 2], F32, tag="wnorm")
            nc.scalar.activation(wnorm_sb[:, 0:1], wsA[:], AF.Sqrt)
            nc.scalar.activation(wnorm_sb[0:72, 1:2], wsB[:], AF.Sqrt)
            winv_sb = sbuf.tile([128, 2], F32, tag="winv")
            nc.vector.reciprocal(winv_sb[:, 0:1], wnorm_sb[:, 0:1])
            nc.vector.reciprocal(winv_sb[0:72, 1:2], wnorm_sb[0:72, 1:2])
            winvrow_sb = sbuf.tile([1, NM], F32, tag="winvrow")
            wr1 = psum.tile([1, 128], F32, tag="ep", name="wr1")
            nc.tensor.transpose(wr1[:], winv_sb[:, 0:1], idn[:])
            nc.vector.tensor_copy(winvrow_sb[:, 0:128], wr1[:])
            wr2 = psum.tile([1, 72], F32, tag="ep", name="wr2")
            nc.tensor.transpose(wr2[:], winv_sb[0:72, 1:2], idn[0:72, 0:72])
            nc.vector.tensor_copy(winvrow_sb[:, 128:NM], wr2[:])
            wbps = psum.tile([RB, NM], F32, tag="ep", name="wbps")
            nc.tensor.matmul(wbps[:], ones1[:], winvrow_sb[:], start=True, stop=True)
            winvbS_sb = sbuf.tile([RB, NM], F32, tag="winvbS")
            nc.scalar.mul(winvbS_sb[:], wbps[:], S_SCALE)
            maskSM_sb = sbuf.tile([RB, NM], F32, tag="maskSM")
            nc.vector.tensor_scalar(maskSM_sb[:], ep_sb[:, 0:NM],
                                    ep_sb[:, NM:NM + 1], None, ALU.is_equal)
            nc.vector.tensor_scalar(maskSM_sb[:], maskSM_sb[:], S_SCALE * M_MARGIN,
                                    None, ALU.mult)

            # ---------------- step 2: k2T_s[o, n] for the local o-slice ----------------
            k2_sb = sbuf.tile([128, 2, 3, NM], BF16, tag="k2")
            for oc in range(2):
                # bank A holds s=0,1 (N=400), bank B holds s=2 (N=200)
                psA = psum.tile([128, 2 * NM], F32, tag="ps2A", bufs=2, name=f"ps2A_{oc}")
                psB = psum.tile([128, NM], F32, tag="ps2B", bufs=2, name=f"ps2B_{oc}")
                n_it = 0
                for h in range(2):          # gathered half h first
                    for g in range(N_CORES):
                        it = 2 * g + h      # global 128-j tile index
                        first = (n_it == 0)
                        last = (n_it == 2 * N_CORES - 1)
                        kv = k1f[:, h].rearrange("p c t n -> p c (t n)")
                        l0 = wcT_sb[:, it, 0, oc, :]
                        l1 = wcT_sb[:, it, 1, oc, :]
                        l2 = wcT_sb[:, it, 2, oc, :]
                        # dt=1: t'=0,1 -> s=0,1 (A[0:400])
                        nc.tensor.matmul(psA[:, 0:2 * NM], l1, kv[:, g, 0:2 * NM],
                                         start=first, stop=False)
                        # dt=0: t'=0 -> s=1 (A[200:400])
                        nc.tensor.matmul(psA[:, NM:2 * NM], l0, kv[:, g, 0:NM],
                                         start=False, stop=False)
                        # dt=2: t'=1,2 -> s=0,1 (A[0:400])
                        nc.tensor.matmul(psA[:, 0:2 * NM], l2, kv[:, g, NM:3 * NM],
                                         start=False, stop=last)
                        # dt=0: t'=1 -> s=2 (B)
                        nc.tensor.matmul(psB[:], l0, kv[:, g, NM:2 * NM],
                                         start=first, stop=False)
                        # dt=1: t'=2 -> s=2 (B)
                        nc.tensor.matmul(psB[:], l1, kv[:, g, 2 * NM:3 * NM],
                                         start=False, stop=last)
                        n_it += 1
                nc.vector.tensor_scalar(k2_sb[:, oc, 0, :], psA[:, 0:NM],
                                        sm_sb[:, 2 + oc:3 + oc], 0.0, ALU.add, ALU.max)
                nc.vector.tensor_scalar(k2_sb[:, oc, 1, :], psA[:, NM:2 * NM],
                                        sm_sb[:, 2 + oc:3 + oc], 0.0, ALU.add, ALU.max)
                nc.vector.tensor_scalar(k2_sb[:, oc, 2, :], psB[:],
                                        sm_sb[:, 2 + oc:3 + oc], 0.0, ALU.add, ALU.max)

            # ---------------- cls partial: [64, 200] ----------------
            cps = psum.tile([BS, NM], F32, tag="ep", name="cps")
            first = True
            for h in range(2):
                for t in range(3):
                    lhs = gbuf16[:, 3 * h + t::6]
                    nc.tensor.matmul(cps[:], lhs[:, 0:BS], k2_sb[:, h, t, :],
                                     start=first, stop=(h == 1 and t == 2))
                    first = False
            clsp_sb = sbuf.tile([BS, NM], F32, tag="clsp")
            nc.vector.tensor_copy(clsp_sb[:], cps[:])

            # ---------------- AllToAll + local tree reduce ----------------
            # Core c ends with the full 8-way sum of batch rows 8c..8c+8.
            cls_bounce = dram.tile([BS, NM], F32, name="cls_bounce")
            cls_x = dram.tile([BS, NM], F32, name="cls_x")
            nc.scalar.dma_start(cls_bounce[:], clsp_sb[:])
            nc.gpsimd.collective_compute(
                "AllToAll", ALU.bypass,
                replica_groups=[list(range(N_CORES))],
                ins=[cls_bounce.opt()], outs=[cls_x.opt()],
            )
            xs_sb = sbuf.tile([RB, N_CORES, NM], F32, tag="xs")
            nc.sync.dma_start(xs_sb[:], cls_x.rearrange("(s p) n -> p s n", p=RB))
            t4_sb = sbuf.tile([RB, 4, NM], F32, tag="t4")
            nc.vector.tensor_tensor(t4_sb[:], xs_sb[:, 0:4], xs_sb[:, 4:8], ALU.add)
            t2_sb = sbuf.tile([RB, 2, NM], F32, tag="t2")
            nc.vector.tensor_tensor(t2_sb[:], t4_sb[:, 0:2], t4_sb[:, 2:4], ALU.add)
            cls_sb = sbuf.tile([RB, NM], F32, tag="cls")
            # fold b_sp into the last tree level: (a + b_sp) + b
            nc.vector.scalar_tensor_tensor(cls_sb[:], t2_sb[:, 0],
                                           ep_sb[:, NM + 1:NM + 2], t2_sb[:, 1],
                                           ALU.add, ALU.add)

            # ---------------- CosFace epilogue ----------------
            # norm path (ACT) and transpose/cos path (PE) run in parallel.
            sq_sb = sbuf.tile([RB, NM], F32, tag="sq")
            ss_sb = sbuf.tile([RB, 1], F32, tag="ss")
            nc.scalar.activation(sq_sb[:], cls_sb[:], AF.Square, accum_out=ss_sb[:])
            rt_sb = sbuf.tile([RB, 1], F32, tag="rt")
            nc.scalar.activation(rt_sb[:], ss_sb[:], AF.Sqrt)
            invx_sb = sbuf.tile([RB, 1], F32, tag="invx")
            nc.vector.reciprocal(invx_sb[:], rt_sb[:])
            clsT_sb = sbuf.tile([128, 2, RB], F32, tag="clsT")
            tp1 = psum.tile([128, RB], F32, tag="ep", name="tp1")
            nc.tensor.transpose(tp1[:], cls_sb[:, 0:128], idn[0:RB, 0:RB])
            nc.vector.tensor_copy(clsT_sb[0:128, 0, :], tp1[:])
            tp2 = psum.tile([72, RB], F32, tag="ep", name="tp2")
            nc.tensor.transpose(tp2[:], cls_sb[:, 128:NM], idn[0:RB, 0:RB])
            nc.vector.tensor_copy(clsT_sb[0:72, 1, :], tp2[:])
            cos_ps = psum.tile([RB, NM], F32, tag="ep", name="cos_ps")
            nc.tensor.matmul(cos_ps[:], clsT_sb[0:128, 0, :], wcls_sb[0:128, 0, :],
                             start=True, stop=False)
            nc.tensor.matmul(cos_ps[:], clsT_sb[0:72, 1, :], wcls_sb[0:72, 1, :],
                             start=False, stop=True)
            t1_sb = sbuf.tile([RB, NM], F32, tag="t1")
            nc.vector.scalar_tensor_tensor(t1_sb[:], cos_ps[:], invx_sb[:],
                                           winvbS_sb[:], ALU.mult, ALU.mult)
            out_sb = sbuf.tile([RB, NM], F32, tag="out")
            nc.vector.tensor_tensor(out_sb[:], t1_sb[:], maskSM_sb[:], ALU.subtract)
            nc.sync.dma_start(Y[:], out_sb[:])

    nc.compile()
    return nc


def _prep_inputs(feat, label, mem_feat, wt, bt, wc, bc, w_sp, b_sp, w_cls):
    bf = ml_dtypes.bfloat16
    f32 = np.float32
    feat = np.ascontiguousarray(np.asarray(feat, dtype=f32))
    mem_feat = np.asarray(mem_feat, dtype=f32)
    wt = np.asarray(wt, dtype=f32)
    bt = np.asarray(bt, dtype=f32)
    wc = np.asarray(wc, dtype=f32)
    bc = np.asarray(bc, dtype=f32)
    w_sp = np.asarray(w_sp, dtype=f32)
    b_sp = np.asarray(b_sp, dtype=f32)
    w_cls = np.asarray(w_cls, dtype=f32)
    label = np.asarray(label)

    # V[u, t]: shifted copies of w_sp so conv+sp_down folds into G
    V = np.zeros((HW, 3), f32)
    V[:HW - 1, 0] = w_sp[0, 1:]
    V[:, 1] = w_sp[0, :]
    V[1:, 2] = w_sp[0, :HW - 1]
    vm = np.zeros((128, 2, 3), f32)
    vm[0:128, 0, :] = V[0:128]
    vm[0:68, 1, :] = V[128:HW]
    vm = vm.astype(bf)

    # mft[p, it, n] = mem_feat.T[it*128+p, n]
    mft = np.ascontiguousarray(
        mem_feat.T.reshape(NIT, 128, NM).transpose(1, 0, 2)).astype(bf)

    # wcls[p, half, n]: wclsT rows 0:128 | rows 128:200 (zero-padded)
    wclsT = np.ascontiguousarray(w_cls.T)          # [200, 200] f32
    wcls = np.zeros((128, 2, NM), f32)
    wcls[0:128, 0] = wclsT[0:128]
    wcls[0:72, 1] = wclsT[128:NM]

    iota = np.arange(NM, dtype=f32)

    fv = feat.reshape(BS, C, HW)
    in_maps = []
    for c in range(N_CORES):
        J = slice(c * SH, (c + 1) * SH)
        # wt[p, jc, t, it, jl] = wt[it*128+p, c*256+jc*128+jl, t]
        wt_c = np.ascontiguousarray(
            wt[:, J, :].reshape(NIT, 128, 2, 128, 3).transpose(1, 2, 4, 0, 3)
        ).astype(bf)
        # wct[p, it, dt, oc, ol] = wc[c*256+oc*128+ol, it*128+p, dt]
        wc_c = np.ascontiguousarray(
            wc[J, :, :].reshape(2, 128, NIT, 128, 3).transpose(3, 2, 4, 0, 1)
        ).astype(bf)
        # featT slice: [u, x] with x = b*256 + i_local
        ft_c = np.ascontiguousarray(
            fv[:, J, :].transpose(2, 0, 1).reshape(HW, NX)).astype(bf)
        ft0 = np.ascontiguousarray(ft_c[0:128])
        ft1 = np.zeros((68, NX), f32).astype(bf)
        ft1[0:68] = ft_c[128:HW]
        sm = np.zeros((128, 4), f32)
        sm[:, 0] = bt[J][0:128]
        sm[:, 1] = bt[J][128:256]
        sm[:, 2] = bc[J][0:128]
        sm[:, 3] = bc[J][128:256]
        ep = np.zeros((RB, NM + 2), f32)
        ep[:, 0:NM] = iota[None, :]
        ep[:, NM] = label[c * RB:(c + 1) * RB].astype(f32)
        ep[:, NM + 1] = b_sp[0]
        in_maps.append({
            "mft": mft, "wtc": wt_c, "wct": wc_c,
            "ft0": ft0, "ft1": ft1, "vm": vm,
            "sm": sm, "wcls": wcls, "ep": ep,
        })
    return in_maps


def kernel(**inputs) -> np.ndarray:
    global LAST_RESULT
    if "nc" not in _CACHE:
        _CACHE["nc"] = build_nc()
    nc = _CACHE["nc"]
    in_maps = _prep_inputs(**inputs)
    try:
        res = bass_utils.run_bass_kernel_spmd(
            nc, in_maps, core_ids=list(range(N_CORES)),
            trace=TRACE, **TRACE_KW,
        )
    except Exception:
        # transient NRT/device hiccups recover on retry
        res = bass_utils.run_bass_kernel_spmd(
            nc, in_maps, core_ids=list(range(N_CORES)),
            trace=TRACE, **TRACE_KW,
        )
    LAST_RESULT = res
    return np.concatenate(
        [np.asarray(res.results[c]["y"], dtype=np.float32) for c in range(N_CORES)],
        axis=0,
    )


# revision 25
# speedup vs baseline: 1.0675x; 1.0514x over previous
"""Trainium2 Bass kernel for nn_DimCosSoftmaxModule (8-core SPMD).

Math (exact refactor of the reference):
  k1[n,j,t] = relu(sum_i mem_feat[n,i] wt[i,j,t] + bt[j])                 [200,2048,3]
  k2[n,o,s] = relu(sum_{i,dt} wc[o,i,dt] k1pad[n,i,s+dt-1] + bc[o])      [200,2048,3]
  conv/sp_down fold: cls[b,n] = sum_{i,t} G[b,i,t] k2[n,i,t] + b_sp
      where G[b,i,t] = sum_u feat[b,i,u] V[u,t],  V = shifted copies of w_sp
  out = 30*(cosine(cls, w_cls) - 0.5*onehot(label))

Sharding: tensor-parallel over the 2048 channel dim (256 ch/core).
  step1 column-sharded -> single AllGather of k1 (bf16, Shared output)
  -> step2 o-sharded -> local partial cls -> AllToAll (mesh, ~8us) +
  DVE tree-sum so core c keeps batch rows 8c..8c+8 -> row-local
  CosFace.  Host reassembles the 8 row-shards of y.

Measured constraints this schedule is built around: no collective
transfer starts before the device-wide collective-init barrier ends
(~60us) plus ~11.5us first-op setup; a single RDH AllGather moves
2.45MB in ~28us (fastest per byte of AG/RS/AllToAll here); the tail
AllToAll+local-sum replaces a ReduceScatter at half the latency.

All bulk inputs are host-restaged to fully-contiguous [128, L] partition
layouts so every HWDGE DMA moves multi-KB runs per partition (line rate).
"""
import os
os.environ.setdefault("NEURON_RT_DBG_RDH_CC", "0")

import numpy as np
import ml_dtypes

import concourse.bass as bass
import concourse.bacc as bacc
import concourse.mybir as mybir
import concourse.tile as tile
from concourse import bass_utils
from concourse.masks import make_identity

N_CORES = 8
BS, C, HW = 64, 2048, 196
NM = 200                 # N_MEM == NUM_CLASSES
SH = C // N_CORES        # 256 channels per core
NIT = C // 128           # 16 i-tiles of 128
NOT = C // 128           # 16 o-tiles of 128 (step2 partial covers all o)
RB = BS // N_CORES       # rows per core after the final exchange
S_SCALE, M_MARGIN = 30.0, 0.5

BF16 = mybir.dt.bfloat16
FP16 = mybir.dt.float16
F32 = mybir.dt.float32
AF = mybir.ActivationFunctionType
ALU = mybir.AluOpType

NX = BS * SH             # 16384 G columns per core
NQ = NX // 128           # 128 q-groups

# x-column chunk boundaries for ft/G pipelining (3 chunks per u-half)
FT_SPLITS = [0, 43 * 128, 86 * 128, NX]

TRACE = False
TRACE_KW = {}
LAST_RESULT = None
_CACHE = {}


def build_nc():
    nc = bacc.Bacc("TRN2", target_bir_lowering=False, debug=False, num_devices=N_CORES)

    # per-core external inputs; every bulk tensor is staged host-side so the
    # DMA is identity-contiguous per partition.
    MFT = nc.dram_tensor("mft", [128, NIT, NM], BF16, kind="ExternalInput")
    WT = nc.dram_tensor("wtc", [128, 2, 3, NIT, 128], BF16, kind="ExternalInput")
    WCT = nc.dram_tensor("wct", [128, NIT, 3, 2, 128], BF16, kind="ExternalInput")
    FT0 = nc.dram_tensor("ft0", [128, NX], BF16, kind="ExternalInput")
    FT1 = nc.dram_tensor("ft1", [68, NX], BF16, kind="ExternalInput")
    VM = nc.dram_tensor("vm", [128, 2, 3], BF16, kind="ExternalInput")
    SM = nc.dram_tensor("sm", [128, 4], F32, kind="ExternalInput")     # bt | bc
    WCLS = nc.dram_tensor("wcls", [128, 2, NM], F32, kind="ExternalInput")
    EP = nc.dram_tensor("ep", [RB, NM + 2], F32, kind="ExternalInput") # iota | lbl | bsp
    Y = nc.dram_tensor("y", [RB, NM], F32, kind="ExternalOutput")

    with tile.TileContext(nc) as tc:
        with (
            tc.tile_pool(name="sbuf", bufs=1) as sbuf,
            tc.tile_pool(name="psum", bufs=1, space="PSUM") as psum,
            tc.tile_pool(name="dram", bufs=1, space="DRAM") as dram,
        ):
            # ---------------- input DMAs ----------------
            # sync(SP) ring:   mft -> wt jc1 -> wct ot 0:8 -> ft1 chunks
            # scalar(ACT) ring: wt jc0 -> wct ot 8:16 -> smalls -> ft0 chunks
            # The FIRST 8 DMAs emitted map 1:1 onto the 8 HWDGE semaphore
            # lanes; later DMAs alias those lanes.  Keeping every step-1/2
            # critical input in the first 8 means no critical wait ever
            # aliases a big late-finishing ft transfer.
            mf_sb = sbuf.tile([128, NIT, NM], BF16, tag="mf")
            nc.sync.dma_start(mf_sb[:], MFT[:])
            # all small tensors lead the scalar ring: they gate cheap DVE/ACT
            # ops (relu biases, masks) whose queue position would otherwise
            # head-of-line-block the whole engine behind bulk transfers
            sm_sb = sbuf.tile([128, 4], F32, tag="sm")
            nc.scalar.dma_start(sm_sb[:], SM[:])
            v_sb = sbuf.tile([128, 2, 3], BF16, tag="v")
            nc.scalar.dma_start(v_sb[:], VM[:])
            wcls_sb = sbuf.tile([128, 2, NM], F32, tag="wcls")
            nc.scalar.dma_start(wcls_sb[:], WCLS[:])
            ep_sb = sbuf.tile([RB, NM + 2], F32, tag="ep")
            nc.scalar.dma_start(ep_sb[:], EP[:])
            wt_sb = sbuf.tile([128, 2, 3, NIT, 128], BF16, tag="wt")
            nc.scalar.dma_start(wt_sb[:, 0], WT[:, 0])
            nc.sync.dma_start(wt_sb[:, 1], WT[:, 1])

            # k1 bounce buffers ride the rings BEFORE wct/ft so the AllGather
            # halves trigger as soon as step 1 finishes
            k1_sb = sbuf.tile([128, 2, 3, NM], BF16, tag="k1")
            kb = dram.tile([SH, 3, NM], BF16, name="k1_bounce")
            kg = dram.tile([C, 3, NM], BF16, name="k1_gath", addr_space="Shared")
            wcT_sb = sbuf.tile([128, NIT, 3, 2, 128], BF16, tag="wcT")
            nc.sync.dma_start(wcT_sb[:, 0:8], WCT[:, 0:8])
            nc.scalar.dma_start(wcT_sb[:, 8:16], WCT[:, 8:16])

            ft0_sb = sbuf.tile([128, NX], BF16, tag="ft0")
            ft1_sb = sbuf.tile([68, NX], BF16, tag="ft1")

            # ---------------- constants ----------------
            idn = sbuf.tile([128, 128], F32, tag="idn")
            make_identity(nc, idn[:])
            ones1 = sbuf.tile([1, RB], F32, tag="ones1")
            nc.vector.memset(ones1[:], 1.0)
            onesc = sbuf.tile([128, 1], F32, tag="onesc")
            nc.vector.memset(onesc[:], 1.0)

            # ---------------- step 1 + pipelined AllGather halves ----------------
            # jc-outer so each 128-wide j-half is gathered while the other
            # half computes.  Gathered half jc holds global j-tiles {2c + jc}.
            for jc in range(2):
                for t in range(3):
                    ps1 = psum.tile([128, NM], F32, tag="ps1", bufs=1, name=f"ps1_{jc}_{t}")
                    for it in range(NIT):
                        nc.tensor.matmul(
                            ps1[:],
                            wt_sb[:, jc, t, it, :],
                            mf_sb[:, it, :],
                            start=(it == 0), stop=(it == NIT - 1),
                        )
                    nc.vector.tensor_scalar(k1_sb[:, jc, t, :], ps1[:],
                                            sm_sb[:, jc:jc + 1], 0.0,
                                            ALU.add, ALU.max)
            nc.sync.dma_start(kb.rearrange("(jc p) t n -> p jc t n", p=128), k1_sb[:])
            nc.gpsimd.collective_compute(
                "AllGather", ALU.bypass,
                replica_groups=[list(range(N_CORES))],
                ins=[kb.opt()], outs=[kg.opt()],
            )

            # ft (for G) and the k1f reloads, emitted after the AG trigger so
            # the reloads' dependency on the gathered output is tracked
            for a, b in zip(FT_SPLITS[:-1], FT_SPLITS[1:]):
                nc.sync.dma_start(ft1_sb[:, a:b], FT1[:, a:b])
                nc.sync.dma_start(ft0_sb[:, a:b], FT0[:, a:b])
            k1f = sbuf.tile([128, 2, N_CORES, 3, NM], BF16, tag="k1f")
            kgv = kg.rearrange("(c jc p) t n -> p jc c t n", p=128, jc=2)
            nc.sync.dma_start(k1f[:, 0, 0:4], kgv[:, 0, 0:4])
            nc.scalar.dma_start(k1f[:, 0, 4:8], kgv[:, 0, 4:8])
            nc.sync.dma_start(k1f[:, 1, 0:4], kgv[:, 1, 0:4])
            nc.scalar.dma_start(k1f[:, 1, 4:8], kgv[:, 1, 4:8])

            # ---------------- G: featT-stationary matmuls ----------------
            # out[x-chunk, t] = sum_u featT[u, x] V[u, t]; 42 chunks packed per
            # PSUM bank, one DVE cast-copy per bank into gbuf16.
            # gbuf16 free index = 3*q + t with q = chunk = b*2 + h.
            gbuf16 = sbuf.tile([128, NQ * 3], BF16, tag="gbuf16")
            CPB = 42                       # chunks per bank
            nbanks = (NQ + CPB - 1) // CPB
            for bank in range(nbanks):
                c0 = bank * CPB
                c1 = min(c0 + CPB, NQ)
                gpk = psum.tile([128, CPB * 3], F32, tag="gpk", bufs=2, name=f"gpk{bank}")
                for c in range(c0, c1):
                    col = (c - c0) * 3
                    nc.tensor.matmul(gpk[:, col:col + 3],
                                     ft0_sb[:, c * 128:(c + 1) * 128],
                                     v_sb[0:128, 0, :], start=True, stop=False)
                    nc.tensor.matmul(gpk[:, col:col + 3],
                                     ft1_sb[0:68, c * 128:(c + 1) * 128],
                                     v_sb[0:68, 1, :], start=False, stop=True)
                nc.vector.tensor_copy(gbuf16[:, c0 * 3:c1 * 3], gpk[:, 0:(c1 - c0) * 3])

            # ---------------- CosFace precompute (overlaps collective) ----------------
            # S / |w_cls_col| broadcast to [RB, NM], and S*M*onehot mask.
            wsq_sb = sbuf.tile([128, 2, NM], F32, tag="wsq")
            nc.scalar.activation(wsq_sb[0:128, 0, :], wcls_sb[0:128, 0, :], AF.Square)
            nc.scalar.activation(wsq_sb[0:72, 1, :], wcls_sb[0:72, 1, :], AF.Square)
            wsA = psum.tile([128, 1], F32, tag="ep", name="wsA")
            nc.tensor.matmul(wsA[:], wsq_sb[0:128, 0, 0:128], onesc[0:128, :], start=True, stop=False)
            nc.tensor.matmul(wsA[:], wsq_sb[0:72, 1, 0:128], onesc[0:72, :], start=False, stop=True)
            wsB = psum.tile([72, 1], F32, tag="ep", name="wsB")
            nc.tensor.matmul(wsB[:], wsq_sb[0:128, 0, 128:NM], onesc[0:128, :], start=True, stop=False)
            nc.tensor.matmul(wsB[:], wsq_sb[0:72, 1, 128:NM], onesc[0:72, :], start=False, stop=True)
            wnorm_sb = sbuf.tile([128, 2], F32, tag="wnorm")
            nc.scalar.activation(wnorm_sb[:, 0:1], wsA[:], AF.Sqrt)
            nc.scalar.activation(wnorm_sb[0:72, 1:2], wsB[:], AF.Sqrt)
            winv_sb = sbuf.tile([128, 2], F32, tag="winv")
            nc.vector.reciprocal(winv_sb[:, 0:1], wnorm_sb[:, 0:1])
            nc.vector.reciprocal(winv_sb[0:72, 1:2], wnorm_sb[0:72, 1:2])
            winvrow_sb = sbuf.tile([1, NM], F32, tag="winvrow")
            wr1 = psum.tile([1, 128], F32, tag="ep", name="wr1")
            nc.tensor.transpose(wr1[:], winv_sb[:, 0:1], idn[:])
            nc.vector.tensor_copy(winvrow_sb[:, 0:128], wr1[:])
            wr2 = psum.tile([1, 72], F32, tag="ep", name="wr2")
            nc.tensor.transpose(wr2[:], winv_sb[0:72, 1:2], idn[0:72, 0:72])
            nc.vector.tensor_copy(winvrow_sb[:, 128:NM], wr2[:])
            wbps = psum.tile([RB, NM], F32, tag="ep", name="wbps")
            nc.tensor.matmul(wbps[:], ones1[:], winvrow_sb[:], start=True, stop=True)
            winvbS_sb = sbuf.tile([RB, NM], F32, tag="winvbS")
            nc.scalar.mul(winvbS_sb[:], wbps[:], S_SCALE)
            maskSM_sb = sbuf.tile([RB, NM], F32, tag="maskSM")
            nc.vector.tensor_scalar(maskSM_sb[:], ep_sb[:, 0:NM],
                                    ep_sb[:, NM:NM + 1], None, ALU.is_equal)
            nc.vector.tensor_scalar(maskSM_sb[:], maskSM_sb[:], S_SCALE * M_MARGIN,
                                    None, ALU.mult)

            # ---------------- step 2: k2T_s[o, n] for the local o-slice ----------------
            k2_sb = sbuf.tile([128, 2, 3, NM], BF16, tag="k2")
            for oc in range(2):
                # bank A holds s=0,1 (N=400), bank B holds s=2 (N=200)
                psA = psum.tile([128, 2 * NM], F32, tag="ps2A", bufs=2, name=f"ps2A_{oc}")
                psB = psum.tile([128, NM], F32, tag="ps2B", bufs=2, name=f"ps2B_{oc}")
                n_it = 0
                for h in range(2):          # gathered half h first
                    for g in range(N_CORES):
                        it = 2 * g + h      # global 128-j tile index
                        first = (n_it == 0)
                        last = (n_it == 2 * N_CORES - 1)
                        kv = k1f[:, h].rearrange("p c t n -> p c (t n)")
                        l0 = wcT_sb[:, it, 0, oc, :]
                        l1 = wcT_sb[:, it, 1, oc, :]
                        l2 = wcT_sb[:, it, 2, oc, :]
                        # dt=1: t'=0,1 -> s=0,1 (A[0:400])
                        nc.tensor.matmul(psA[:, 0:2 * NM], l1, kv[:, g, 0:2 * NM],
                                         start=first, stop=False)
                        # dt=0: t'=0 -> s=1 (A[200:400])
                        nc.tensor.matmul(psA[:, NM:2 * NM], l0, kv[:, g, 0:NM],
                                         start=False, stop=False)
                        # dt=2: t'=1,2 -> s=0,1 (A[0:400])
                        nc.tensor.matmul(psA[:, 0:2 * NM], l2, kv[:, g, NM:3 * NM],
                                         start=False, stop=last)
                        # dt=0: t'=1 -> s=2 (B)
                        nc.tensor.matmul(psB[:], l0, kv[:, g, NM:2 * NM],
                                         start=first, stop=False)
                        # dt=1: t'=2 -> s=2 (B)
                        nc.tensor.matmul(psB[:], l1, kv[:, g, 2 * NM:3 * NM],
                                         start=False, stop=last)
                        n_it += 1
                nc.vector.tensor_scalar(k2_sb[:, oc, 0, :], psA[:, 0:NM],
                                        sm_sb[:, 2 + oc:3 + oc], 0.0, ALU.add, ALU.max)
                nc.vector.tensor_scalar(k2_sb[:, oc, 1, :], psA[:, NM:2 * NM],
                                        sm_sb[:, 2 + oc:3 + oc], 0.0, ALU.add, ALU.max)
                nc.vector.tensor_scalar(k2_sb[:, oc, 2, :], psB[:],
                                        sm_sb[:, 2 + oc:3 + oc], 0.0, ALU.add, ALU.max)

            # ---------------- cls partial: [64, 200] ----------------
            cps = psum.tile([BS, NM], F32, tag="ep", name="cps")
            first = True
            for h in range(2):
                for t in range(3):
                    lhs = gbuf16[:, 3 * h + t::6]
                    nc.tensor.matmul(cps[:], lhs[:, 0:BS], k2_sb[:, h, t, :],
                                     start=first, stop=(h == 1 and t == 2))
                    first = False
            clsp_sb = sbuf.tile([BS, NM], F32, tag="clsp")
            nc.vector.tensor_copy(clsp_sb[:], cps[:])

            # ---------------- AllToAll + local tree reduce ----------------
            # Core c ends with the full 8-way sum of batch rows 8c..8c+8.
            cls_bounce = dram.tile([BS, NM], F32, name="cls_bounce")
            cls_x = dram.tile([BS, NM], F32, name="cls_x")
            nc.scalar.dma_start(cls_bounce[:], clsp_sb[:])
            nc.gpsimd.collective_compute(
                "AllToAll", ALU.bypass,
                replica_groups=[list(range(N_CORES))],
                ins=[cls_bounce.opt()], outs=[cls_x.opt()],
            )
            xs_sb = sbuf.tile([RB, N_CORES, NM], F32, tag="xs")
            nc.sync.dma_start(xs_sb[:], cls_x.rearrange("(s p) n -> p s n", p=RB))
            t4_sb = sbuf.tile([RB, 4, NM], F32, tag="t4")
            nc.vector.tensor_tensor(t4_sb[:], xs_sb[:, 0:4], xs_sb[:, 4:8], ALU.add)
            t2_sb = sbuf.tile([RB, 2, NM], F32, tag="t2")
            nc.vector.tensor_tensor(t2_sb[:], t4_sb[:, 0:2], t4_sb[:, 2:4], ALU.add)
            cls_sb = sbuf.tile([RB, NM], F32, tag="cls")
            # fold b_sp into the last tree level: (a + b_sp) + b
            nc.vector.scalar_tensor_tensor(cls_sb[:], t2_sb[:, 0],
                                           ep_sb[:, NM + 1:NM + 2], t2_sb[:, 1],
                                           ALU.add, ALU.add)

            # ---------------- CosFace epilogue ----------------
            # norm path (ACT) and transpose/cos path (PE) run in parallel.
            sq_sb = sbuf.tile([RB, NM], F32, tag="sq")
            ss_sb = sbuf.tile([RB, 1], F32, tag="ss")
            nc.scalar.activation(sq_sb[:], cls_sb[:], AF.Square, accum_out=ss_sb[:])
            rt_sb = sbuf.tile([RB, 1], F32, tag="rt")
            nc.scalar.activation(rt_sb[:], ss_sb[:], AF.Sqrt)
            invx_sb = sbuf.tile([RB, 1], F32, tag="invx")
            nc.vector.reciprocal(invx_sb[:], rt_sb[:])
            clsT_sb = sbuf.tile([128, 2, RB], F32, tag="clsT")
            tp1 = psum.tile([128, RB], F32, tag="ep", name="tp1")
            nc.tensor.transpose(tp1[:], cls_sb[:, 0:128], idn[0:RB, 0:RB])
            nc.vector.tensor_copy(clsT_sb[0:128, 0, :], tp1[:])
            tp2 = psum.tile([72, RB], F32, tag="ep", name="tp2")
            nc.tensor.transpose(tp2[:], cls_sb[:, 128:NM], idn[0:RB, 0:RB])
            nc.vector.tensor_copy(clsT_sb[0:72, 1, :], tp2[:])
            cos_ps = psum.tile([RB, NM], F32, tag="ep", name="cos_ps")
            nc.tensor.matmul(cos_ps[:], clsT_sb[0:128, 0, :], wcls_sb[0:128, 0, :],
                             start=True, stop=False)
            nc.tensor.matmul(cos_ps[:], clsT_sb[0:72, 1, :], wcls_sb[0:72, 1, :],
                             start=False, stop=True)
            t1_sb = sbuf.tile([RB, NM], F32, tag="t1")
            nc.vector.scalar_tensor_tensor(t1_sb[:], cos_ps[:], invx_sb[:],
                                           winvbS_sb[:], ALU.mult, ALU.mult)
            out_sb = sbuf.tile([RB, NM], F32, tag="out")
            nc.vector.tensor_tensor(out_sb[:], t1_sb[:], maskSM_sb[:], ALU.subtract)
            nc.sync.dma_start(Y[:], out_sb[:])

    nc.compile()
    return nc


def _prep_inputs(feat, label, mem_feat, wt, bt, wc, bc, w_sp, b_sp, w_cls):
    bf = ml_dtypes.bfloat16
    f32 = np.float32
    feat = np.ascontiguousarray(np.asarray(feat, dtype=f32))
    mem_feat = np.asarray(mem_feat, dtype=f32)
    wt = np.asarray(wt, dtype=f32)
    bt = np.asarray(bt, dtype=f32)
    wc = np.asarray(wc, dtype=f32)
    bc = np.asarray(bc, dtype=f32)
    w_sp = np.asarray(w_sp, dtype=f32)
    b_sp = np.asarray(b_sp, dtype=f32)
    w_cls = np.asarray(w_cls, dtype=f32)
    label = np.asarray(label)

    # V[u, t]: shifted copies of w_sp so conv+sp_down folds into G
    V = np.zeros((HW, 3), f32)
    V[:HW - 1, 0] = w_sp[0, 1:]
    V[:, 1] = w_sp[0, :]
    V[1:, 2] = w_sp[0, :HW - 1]
    vm = np.zeros((128, 2, 3), f32)
    vm[0:128, 0, :] = V[0:128]
    vm[0:68, 1, :] = V[128:HW]
    vm = vm.astype(bf)

    # mft[p, it, n] = mem_feat.T[it*128+p, n]
    mft = np.ascontiguousarray(
        mem_feat.T.reshape(NIT, 128, NM).transpose(1, 0, 2)).astype(bf)

    # wcls[p, half, n]: wclsT rows 0:128 | rows 128:200 (zero-padded)
    wclsT = np.ascontiguousarray(w_cls.T)          # [200, 200] f32
    wcls = np.zeros((128, 2, NM), f32)
    wcls[0:128, 0] = wclsT[0:128]
    wcls[0:72, 1] = wclsT[128:NM]

    iota = np.arange(NM, dtype=f32)

    fv = feat.reshape(BS, C, HW)
    in_maps = []
    for c in range(N_CORES):
        J = slice(c * SH, (c + 1) * SH)
        # wt[p, jc, t, it, jl] = wt[it*128+p, c*256+jc*128+jl, t]
        wt_c = np.ascontiguousarray(
            wt[:, J, :].reshape(NIT, 128, 2, 128, 3).transpose(1, 2, 4, 0, 3)
        ).astype(bf)
        # wct[p, it, dt, oc, ol] = wc[c*256+oc*128+ol, it*128+p, dt]
        wc_c = np.ascontiguousarray(
            wc[J, :, :].reshape(2, 128, NIT, 128, 3).transpose(3, 2, 4, 0, 1)
        ).astype(bf)
        # featT slice: [u, x] with x = b*256 + i_local
        ft_c = np.ascontiguousarray(
            fv[:, J, :].transpose(2, 0, 1).reshape(HW, NX)).astype(bf)
        ft0 = np.ascontiguousarray(ft_c[0:128])
        ft1 = np.zeros((68, NX), f32).astype(bf)
        ft1[0:68] = ft_c[128:HW]
        sm = np.zeros((128, 4), f32)
        sm[:, 0] = bt[J][0:128]
        sm[:, 1] = bt[J][128:256]
        sm[:, 2] = bc[J][0:128]
        sm[:, 3] = bc[J][128:256]
        ep = np.zeros((RB, NM + 2), f32)
        ep[:, 0:NM] = iota[None, :]
        ep[:, NM] = label[c * RB:(c + 1) * RB].astype(f32)
        ep[:, NM + 1] = b_sp[0]
        in_maps.append({
            "mft": mft, "wtc": wt_c, "wct": wc_c,
            "ft0": ft0, "ft1": ft1, "vm": vm,
            "sm": sm, "wcls": wcls, "ep": ep,
        })
    return in_maps


def kernel(**inputs) -> np.ndarray:
    global LAST_RESULT
    if "nc" not in _CACHE:
        _CACHE["nc"] = build_nc()
    nc = _CACHE["nc"]
    in_maps = _prep_inputs(**inputs)
    try:
        res = bass_utils.run_bass_kernel_spmd(
            nc, in_maps, core_ids=list(range(N_CORES)),
            trace=TRACE, **TRACE_KW,
        )
    except Exception:
        # transient NRT/device hiccups recover on retry
        res = bass_utils.run_bass_kernel_spmd(
            nc, in_maps, core_ids=list(range(N_CORES)),
            trace=TRACE, **TRACE_KW,
        )
    LAST_RESULT = res
    return np.concatenate(
        [np.asarray(res.results[c]["y"], dtype=np.float32) for c in range(N_CORES)],
        axis=0,
    )
